# revision 11
# baseline (speedup 1.0000x reference)
"""Trainium2 Bass kernel for nn_HabitatGraph (gnn_message_passing).

Full-input contract: kernel(**inputs) takes the complete arrays, shards the
batch (graph) dimension B=256 across 8 NeuronCores (32 graphs each), runs one
SPMD NEFF via run_bass_kernel_spmd, and gathers the full [256,256,256] output.

Math (reference.py, exploiting that dist_mat is symmetric and >= 0 by
construction, so to_undirected's mean reduces to dist itself):
  sim  = cosine_similarity(x_g)                    # [H,H] per graph
  out  = m_i * m_j * (1-eye) * relu(sim) * exp(-dist^2 / (sigma^2 + EPS))
sigma is a GLOBAL (whole-batch) std over masked dist entries -> three scalar
sums; computed on host and passed in as one broadcast constant.

Device-side structure:
 - 1/sqrt(v) = exp(-0.5*ln(v)): Ln+Exp live in ONE activation table set
   (natural_log_exp_and_others); the table list is patched so the compiler
   can only pick that set -> exactly one ACT_TABLE_LOAD in the kernel.
 - the whole edge mask (m_i & m_j & ~eye) is folded into dist on the host by
   poisoning masked entries with a huge value: exp(-huge^2/sigma^2) == 0.
   No mask tensors on device at all.
 - bf16 end to end; DRAM layouts are partition-major with dist/out pair-
   packed so every DMA moves 2KB-contiguous per-partition rows.
 - engine balance: DVE does xsq/xn/relu/final-mul, GpSimd does the norm-scale
   partition broadcast + dist^2, ACT does only Exp/Ln, PE does norms + gram.
"""

import numpy as np
import ml_dtypes
from contextlib import ExitStack

import concourse.bacc as bacc_mod
from concourse import bacc, bass, mybir, tile
from concourse.bass_utils import run_bass_kernel_spmd

N_CORES = 8
B, H, FEAT = 256, 256, 512
SHARD = B // N_CORES          # 32 graphs per core
KC = FEAT // 128              # 4 k-chunks of the contraction dim
EPS = 1e-6
DIAG_POISON = 1.0e4           # exp(-poison^2/sigma^2) == 0.0 in bf16/f32

F32 = mybir.dt.float32
BF16 = mybir.dt.bfloat16
AF = mybir.ActivationFunctionType
ALU = mybir.AluOpType

_orig_get_tables = bacc_mod.get_activation_tables


def _only_nl_exp_tables(arch):
    """Keep act_func_set indices intact but blank every set except
    natural_log_exp_and_others, so insert_act_table_loads emits exactly one
    table load for our {Ln, Exp} usage."""
    tabs = dict(_orig_get_tables(arch))
    return {
        name: (fns if name == "natural_log_exp_and_others" else set())
        for name, fns in tabs.items()
    }


def build_nc():
    bacc_mod.get_activation_tables = _only_nl_exp_tables
    try:
        nc = bacc.Bacc("TRN2", debug=False, num_devices=N_CORES)

        # partition-major host layouts so each DMA partition row is one
        # contiguous chunk (xt: 2KB; dist/out pair-packed: 2KB).
        xt = nc.dram_tensor("xt", [SHARD, 128, KC, H], BF16, kind="ExternalInput").ap()
        dist = nc.dram_tensor(
            "dist", [SHARD // 2, 128, 2, 2, H], BF16, kind="ExternalInput"
        ).ap()
        scal = nc.dram_tensor("scal", [128, 1], F32, kind="ExternalInput").ap()
        out = nc.dram_tensor(
            "out", [SHARD // 2, 128, 2, 2, H], BF16, kind="ExternalOutput"
        ).ap()

        with tile.TileContext(nc) as tc, ExitStack() as ctx:
            const = ctx.enter_context(tc.tile_pool(name="const", bufs=1))
            xpool = ctx.enter_context(tc.tile_pool(name="x", bufs=8))
            xqpool = ctx.enter_context(tc.tile_pool(name="xq", bufs=3))
            xnpool = ctx.enter_context(tc.tile_pool(name="xn", bufs=3))
            dpool = ctx.enter_context(tc.tile_pool(name="d", bufs=3))
            spool = ctx.enter_context(tc.tile_pool(name="s", bufs=3))
            epool = ctx.enter_context(tc.tile_pool(name="e", bufs=4))
            opool = ctx.enter_context(tc.tile_pool(name="o", bufs=3))
            ps_n = ctx.enter_context(tc.tile_pool(name="psn", bufs=2, space="PSUM"))
            ps_s = ctx.enter_context(tc.tile_pool(name="pss", bufs=3, space="PSUM"))

            scal_t = const.tile([128, 1], F32)
            nc.sync.dma_start(scal_t[:], scal[:])
            ones_t = const.tile([128, 1], BF16)
            nc.vector.memset(ones_t[:], 1.0)
            tiny_t = const.tile([1, 1], F32)
            nc.vector.memset(tiny_t[:], 1e-30)

            for b4 in range(SHARD // 4):
                # ---- phase 1: x loads + squared column norms for 4 graphs
                xts = []
                nrm4 = ps_n.tile([1, 4, H], F32, tag="nrm4")
                for r4 in range(4):
                    g = b4 * 4 + r4
                    xtile = xpool.tile([128, KC, H], BF16, tag="xtile")
                    nc.sync.dma_start(xtile[:], xt[g])
                    xts.append(xtile)
                    xsq = xqpool.tile([128, KC, H], BF16, tag="xsq")
                    nc.vector.tensor_mul(xsq[:], xtile[:], xtile[:])
                    for c in range(KC):
                        nc.tensor.matmul(nrm4[:, r4, :], ones_t[:], xsq[:, c, :],
                                         start=(c == 0), stop=(c == KC - 1))

                # ---- batched rsqrt via Ln/Exp (one table set), then one
                # partition-broadcast of all 4 graphs' scales
                lnv = spool.tile([1, 4, H], F32, tag="lnv")
                nc.scalar.activation(lnv[:], nrm4[:], AF.Ln, bias=tiny_t[:])
                sr4 = spool.tile([1, 4, H], BF16, tag="sr4")
                nc.scalar.activation(sr4[:], lnv[:], AF.Exp, scale=-0.5)
                sful4 = spool.tile([128, 4, H], BF16, tag="sful4")
                nc.gpsimd.partition_broadcast(sful4[:], sr4[:])

                # ---- phase 2: two graph-pairs
                for pr in range(2):
                    gp = b4 * 2 + pr
                    dtile = dpool.tile([128, 2, 2, H], BF16, tag="dtile")
                    nc.sync.dma_start(dtile[:], dist[gp])
                    sqd = dpool.tile([128, 2, 2, H], BF16, tag="sqd")
                    nc.gpsimd.tensor_mul(sqd[:], dtile[:], dtile[:])
                    ew = epool.tile([128, 2, 2, H], BF16, tag="ew")
                    nc.scalar.activation(ew[:], sqd[:], AF.Exp, scale=scal_t[:])

                    otile = opool.tile([128, 2, 2, H], BF16, tag="ot")
                    for j in range(2):
                        r4 = pr * 2 + j
                        sb = sful4[:, r4, :].unsqueeze(1).broadcast_to([128, KC, H])
                        xn = xnpool.tile([128, KC, H], BF16, tag="xn")
                        nc.vector.tensor_mul(xn[:], xts[r4][:], sb)

                        sim = ps_s.tile([128, 2, H], F32, tag="sim")
                        for h in range(2):
                            for c in range(KC):
                                nc.tensor.matmul(
                                    sim[:, h, :],
                                    xn[:, c, h * 128 : (h + 1) * 128],
                                    xn[:, c, :],
                                    start=(c == 0),
                                    stop=(c == KC - 1),
                                )

                        rl = epool.tile([128, 2, H], BF16, tag="rl")
                        nc.vector.tensor_scalar_max(rl[:], sim[:], 0.0)
                        nc.vector.tensor_mul(otile[:, j, :, :], rl[:], ew[:, j, :, :])

                    nc.sync.dma_start(out[gp], otile[:])

        nc.compile()
        return nc
    finally:
        bacc_mod.get_activation_tables = _orig_get_tables


_NC = None


def _get_nc():
    global _NC
    if _NC is None:
        _NC = build_nc()
    return _NC


def make_in_maps(x_feat, dist_mat, mask):
    x = np.asarray(x_feat, np.float32).reshape(B, H, FEAT)
    dist = np.asarray(dist_mat, np.float32)
    mb = np.asarray(mask).astype(bool)

    # global sigma: unbiased std over masked undirected edge weights.
    # pm[b,i,j] = mask_i*mask_j*(1-eye); dist symmetric >= 0 by construction.
    mf64 = mb.astype(np.float64)
    d64 = dist.astype(np.float64)
    k = mf64.sum(1)
    n = float((k * k - k).sum())
    t1 = np.einsum("bij,bj->bi", d64, mf64)
    s1 = float((t1 * mf64).sum()) - float((np.einsum("bii->bi", d64) * mf64).sum())
    d2 = d64 * d64
    t2 = np.einsum("bij,bj->bi", d2, mf64)
    s2 = float((t2 * mf64).sum()) - float((np.einsum("bii->bi", d2) * mf64).sum())
    mean = s1 / max(n, 1.0)
    var = (s2 - n * mean * mean) / max(n - 1.0, 1.0)
    sigma = max(np.sqrt(max(var, 0.0)), EPS)
    neg_inv = np.float32(-1.0 / (sigma * sigma + EPS))

    scal = np.full((128, 1), neg_inv, np.float32)

    # fold the whole edge mask into dist: masked entries (incl. diagonal)
    # get a huge value so exp(-v^2/sigma^2) underflows to exactly 0.
    pm = mb[:, :, None] & mb[:, None, :]
    ii = np.arange(H)
    pm[:, ii, ii] = False
    dmasked = np.where(pm, dist, DIAG_POISON).astype(np.float32)

    in_maps = []
    for c in range(N_CORES):
        sl = slice(c * SHARD, (c + 1) * SHARD)
        # x^T per graph, partition-major: [g, p(128), c(4), h(256)]
        xt = (
            x[sl]
            .transpose(0, 2, 1)              # [32, 512, 256]
            .reshape(SHARD, KC, 128, H)
            .transpose(0, 2, 1, 3)           # [32, 128, 4, 256]
        ).astype(ml_dtypes.bfloat16)
        # dist pair-packed partition-major: [gp(16), p(128), j(2), r(2), h]
        db = (
            dmasked[sl]
            .reshape(SHARD // 2, 2, 2, 128, H)   # [16, j, r, p, h]
            .transpose(0, 3, 1, 2, 4)            # [16, 128, 2, 2, 256]
        ).astype(ml_dtypes.bfloat16)
        in_maps.append(
            {
                "xt": np.ascontiguousarray(xt),
                "dist": np.ascontiguousarray(db),
                "scal": scal,
            }
        )
    return in_maps


def kernel(x_feat, dist_mat, mask):
    nc = _get_nc()
    in_maps = make_in_maps(x_feat, dist_mat, mask)
    res = run_bass_kernel_spmd(nc, in_maps, core_ids=list(range(N_CORES)))
    o = np.concatenate([res.results[c]["out"] for c in range(N_CORES)], axis=0)
    # [128(gp), 128(p), 2(j), 2(r), 256] -> [256, 256, 256] f32
    o = o.transpose(0, 2, 3, 1, 4).reshape(B, H, H)
    return o.astype(np.float32)
